# revision 10
# baseline (speedup 1.0000x reference)
"""DepthCueExtractor TRN2 kernel.

out[b,u,y,x,n] = mean_v(lfi[b,u,y,x,v]) * s_mask[b,n] * h_mask[b,n,y]
  s_mask[b,n]   = sum_{h,w} f_maps[b,h,w,n]
  h_mask[b,n,y] = colsum[b,y,n] / max_w colsum[b,w,n]
  colsum[b,w,n] = sum_h f_maps[b,h,w,n]

Sharding: 8 cores = (batch b in 0..3) x (H-half in 0..1), data-parallel on the
output; cores are fully independent (no collectives -- the 15us constant
overhead of a collective can no longer hide under the shortened DMA stream).
Each core reads its lfi slice (cast f32->bf16 in the DMA) plus the full
f_maps[b] (cast f32->fp8e4m3 in the DMA; the host rotates the W axis per core
so "my" 128 colsum rows are always the first block, keeping the SPMD program
identical across cores). colsum is reduced over the H partition dim with fp8
PE ones-matmuls; the per-sample stats (sum / max over w) use gpsimd partition
reduces. The output phase writes bf16 (rel-err ~2^-9, far inside the 2e-2
gate) and the broadcast multiply is spread across three engines so it hides
under the DMA stream: per u-group of four 1MB tiles, one direct DVE
tensor_mul, two Act-assisted tiles (scalar engine materializes the mlf
broadcast as packed bf16, DVE then runs the mul in 2x_1p mode), and one on
gpsimd. ~47.3MB of DMA traffic per core (4.2 fm + 5.3 lfi + 37.7 store),
~131us of serialized DMA-engine time."""

import numpy as np

import concourse.bass as bass
import concourse.bacc as bacc
import concourse.bass_isa as bass_isa
import concourse.mybir as mybir
import concourse.tile as tile
from concourse.bass_utils import run_bass_kernel_spmd

F32 = mybir.dt.float32
BF16 = mybir.dt.bfloat16
F16 = mybir.dt.float16
FP8 = mybir.dt.float8e4

B, U, H, W, V, N = 4, 9, 256, 256, 9, 64
HY = H // 2


def build_kernel_body(nc, tc, lfi_s, fm, out_s):
    with (
        tc.tile_pool(name="const", bufs=1) as const_pool,
        tc.tile_pool(name="fmp", bufs=1) as fm_pool,
        tc.tile_pool(name="psum", bufs=1, space="PSUM") as psum_pool,
        tc.tile_pool(name="stats", bufs=1) as stats_pool,
        tc.tile_pool(name="lfip", bufs=6) as lfi_pool,
        tc.tile_pool(name="mlfp", bufs=1) as mlf_pool,
        tc.tile_pool(name="outp", bufs=2) as out_pool,
    ):
        ones = const_pool.tile([128, 1], FP8)
        nc.vector.memset(ones[:], 1.0)

        # ---- loads (all on the gpsimd SWDGE queue, which can cast) --------
        # fm in four (w-half, h-half) chunks, f32 -> fp8, so the PE colsum
        # starts after the first 2.9us chunk instead of after the full load
        fmt = {}
        for wc in range(2):
            for h in range(2):
                t = fm_pool.tile([128, HY, N], FP8, name=f"fm{wc}{h}")
                nc.gpsimd.dma_start(
                    out=t[:],
                    in_=fm[h * HY : (h + 1) * HY, wc * HY : (wc + 1) * HY, :],
                )
                fmt[(wc, h)] = t
        # first four lfi u-slices, f32 -> bf16
        lts = []
        for u in range(U):
            lt = lfi_pool.tile([128, W, V], BF16, name=f"lt{u}", tag="lt", bufs=6)
            lts.append(lt)
        for u in range(4):
            nc.gpsimd.dma_start(out=lts[u][:], in_=lfi_s[u])

        # ---- colsum[w, n] = sum_h fm[h, w, n] via fp8 PE ones-matmuls -----
        # one PSUM tile per (w-half, h-half) chunk: self-contained matmuls,
        # h-halves summed on DVE, so the PE starts on chunk 0 right away
        cs_ps = [
            psum_pool.tile([128, N], F32, name=f"cs_ps{i}") for i in range(4)
        ]
        for wc in range(2):
            for h in range(2):
                t = fmt[(wc, h)]
                ps = cs_ps[wc * 2 + h]
                for n in range(N):
                    nc.tensor.matmul(
                        out=ps[:, n : n + 1],
                        lhsT=t[:, :, n],
                        rhs=ones[:, 0:1],
                        start=True,
                        stop=True,
                    )

        # ---- stats: s = sum_w colsum, m = max_w colsum (gpsimd) -----------
        # DVE order: copy csA early, slot the first two lfi reduces in while
        # the PE finishes the B half, then copy csB and finalize.
        mlf = [
            mlf_pool.tile([128, W], F32, name=f"mlf{u}", tag=f"mlf{u}")
            for u in range(U)
        ]

        def reduce_u(u):
            nc.vector.reduce_sum(
                out=mlf[u][:], in_=lts[u][:], axis=mybir.AxisListType.X
            )

        # (DVE may read at most one PSUM operand per instruction)
        tA = stats_pool.tile([128, N], F32)
        nc.vector.tensor_copy(out=tA[:], in_=cs_ps[0][:])
        csA = stats_pool.tile([128, N], F32)
        nc.vector.tensor_add(out=csA[:], in0=tA[:], in1=cs_ps[1][:])
        reduce_u(0)
        reduce_u(1)
        tB = stats_pool.tile([128, N], F32)
        nc.vector.tensor_copy(out=tB[:], in_=cs_ps[2][:])
        csB = stats_pool.tile([128, N], F32)
        nc.vector.tensor_add(out=csB[:], in0=tB[:], in1=cs_ps[3][:])

        raA = stats_pool.tile([128, N], F32)
        nc.gpsimd.partition_all_reduce(raA[:], csA[:], 128, bass_isa.ReduceOp.add)
        rmA = stats_pool.tile([128, N], F32)
        nc.gpsimd.partition_all_reduce(rmA[:], csA[:], 128, bass_isa.ReduceOp.max)
        for u in range(4, 6):
            nc.gpsimd.dma_start(out=lts[u][:], in_=lfi_s[u])
        raB = stats_pool.tile([128, N], F32)
        nc.gpsimd.partition_all_reduce(raB[:], csB[:], 128, bass_isa.ReduceOp.add)
        rmB = stats_pool.tile([128, N], F32)
        nc.gpsimd.partition_all_reduce(rmB[:], csB[:], 128, bass_isa.ReduceOp.max)
        for u in range(6, U):
            nc.gpsimd.dma_start(out=lts[u][:], in_=lfi_s[u])

        # ---- finalize wf[y, n] = colsum[y,n] * s[n] / (max[n] * V) --------
        s_all = stats_pool.tile([128, N], F32)
        nc.vector.tensor_add(out=s_all[:], in0=raA[:], in1=raB[:])
        m_all = stats_pool.tile([128, N], F32)
        nc.vector.tensor_max(out=m_all[:], in0=rmA[:], in1=rmB[:])
        m9 = stats_pool.tile([128, N], F32)
        nc.vector.tensor_scalar_mul(m9[:], m_all[:], float(V))
        rec = stats_pool.tile([128, N], F32)
        nc.vector.reciprocal(out=rec[:], in_=m9[:])
        sn = stats_pool.tile([128, N], F32)
        nc.vector.tensor_mul(out=sn[:], in0=s_all[:], in1=rec[:])
        wf = stats_pool.tile([128, N], F32)
        nc.vector.tensor_mul(out=wf[:], in0=csA[:], in1=sn[:])
        wfb = stats_pool.tile([128, N], F16)
        nc.vector.tensor_copy(out=wfb[:], in_=wf[:])

        # ---- output phase -------------------------------------------------

        xw = 64

        def mlf_bcast(u, x0, xn):
            msl = mlf[u][:, x0 : x0 + xn]
            return bass.AP(
                tensor=msl.tensor, offset=msl.offset, ap=list(msl.ap) + [[0, N]]
            )

        def wf_bcast(t, xn):
            return bass.AP(
                tensor=t.tensor, offset=t.offset, ap=[t.ap[0], [0, xn], t.ap[1]]
            )

        def store(u, x0, xn, ot):
            nc.sync.dma_start(out=out_s[u, :, x0 : x0 + xn, :], in_=ot[:])

        def emit_direct(u, x0, xn=xw):
            ot = out_pool.tile(
                [128, xn, N], BF16, name=f"otd{u}_{x0}", tag=f"otd{xn}", bufs=3
            )
            nc.vector.tensor_mul(
                out=ot[:], in0=mlf_bcast(u, x0, xn), in1=wf_bcast(wf, xn)
            )
            store(u, x0, xn, ot)

        def emit_act_copy(u, x0):
            mr = out_pool.tile([128, xw, N], F16, name=f"mr{u}_{x0}", tag="mr", bufs=3)
            nc.scalar.copy(out=mr[:], in_=mlf_bcast(u, x0, xw))
            return mr

        def emit_act_mul(u, x0, mr):
            ot = out_pool.tile([128, xw, N], BF16, name=f"ota{u}_{x0}", tag="ota", bufs=3)
            nc.vector.tensor_mul(out=ot[:], in0=mr[:], in1=wf_bcast(wfb, xw))
            store(u, x0, xw, ot)

        def emit_pool(u, x0):
            ot = out_pool.tile([128, xw, N], BF16, name=f"otp{u}_{x0}", tag="otp", bufs=3)
            nc.gpsimd.tensor_mul(
                out=ot[:], in0=mlf_bcast(u, x0, xw), in1=wf_bcast(wf, xw)
            )
            store(u, x0, xw, ot)

        for u in range(U):
            mr1 = emit_act_copy(u, 1 * xw)
            mr2 = emit_act_copy(u, 2 * xw)
            emit_pool(u, 3 * xw)
            if u + 2 < U:
                reduce_u(u + 2)
            if u == 0:
                # 0.5MB first tiles so the store stream starts ~1.5us sooner
                emit_direct(0, 0, 32)
                emit_direct(0, 32, 32)
            else:
                emit_direct(u, 0)
            emit_act_mul(u, 1 * xw, mr1)
            emit_act_mul(u, 2 * xw, mr2)


def build_nc():
    nc = bacc.Bacc("TRN2", target_bir_lowering=False, debug=True)
    lfi_s = nc.dram_tensor("lfi_s", [U, HY, W, V], F32, kind="ExternalInput")
    fm = nc.dram_tensor("fm", [H, W, N], F32, kind="ExternalInput")
    out_s = nc.dram_tensor("out_s", [U, HY, W, N], BF16, kind="ExternalOutput")
    with tile.TileContext(nc) as tc:
        build_kernel_body(nc, tc, lfi_s, fm, out_s)
    nc.compile()
    return nc


_CACHE = {}


def make_in_maps(lfi, f_maps):
    in_maps = []
    for c in range(8):
        b, half = divmod(c, 2)
        lf = np.ascontiguousarray(lfi[b, :, half * HY : (half + 1) * HY])
        fb = f_maps[b]
        if half == 0:
            fmc = np.ascontiguousarray(fb)
        else:
            # rotate W so this core's own colsum rows are the first 128
            fmc = np.ascontiguousarray(
                np.concatenate([fb[:, HY:, :], fb[:, :HY, :]], axis=1)
            )
        in_maps.append({"lfi_s": lf, "fm": fmc})
    return in_maps


def kernel(lfi, f_maps):
    lfi = np.asarray(lfi, dtype=np.float32)
    f_maps = np.asarray(f_maps, dtype=np.float32)
    if "nc" not in _CACHE:
        _CACHE["nc"] = build_nc()
    nc = _CACHE["nc"]
    res = run_bass_kernel_spmd(nc, make_in_maps(lfi, f_maps), list(range(8)))
    out = np.empty((B, U, H, W, N), np.float32)
    for c in range(8):
        b, half = divmod(c, 2)
        out[b, :, half * HY : (half + 1) * HY] = np.asarray(
            res.results[c]["out_s"]
        ).astype(np.float32)
    return out


# revision 15
# speedup vs baseline: 1.1031x; 1.1031x over previous
"""DepthCueExtractor TRN2 kernel.

out[b,u,y,x,n] = mean_v(lfi[b,u,y,x,v]) * s_mask[b,n] * h_mask[b,n,y]
  s_mask[b,n]   = sum_{h,w} f_maps[b,h,w,n]
  h_mask[b,n,y] = colsum[b,y,n] / max_w colsum[b,w,n]
  colsum[b,w,n] = sum_h f_maps[b,h,w,n]

Sharding: 8 cores = (batch b in 0..3) x (H-half in 0..1), data-parallel on the
output; cores are fully independent (no collectives -- the 15us constant
overhead of a collective can no longer hide under the shortened DMA stream).
Each core reads its lfi slice (cast f32->bf16 in the DMA) plus the full
f_maps[b] (cast f32->fp8e4m3 in the DMA; the host rotates the W axis per core
so "my" 128 colsum rows are always the first block, keeping the SPMD program
identical across cores). colsum is reduced over the H partition dim with fp8
PE ones-matmuls; the per-sample stats (sum / max over w) use gpsimd partition
reduces. The output phase writes bf16 (rel-err ~2^-9, far inside the 2e-2
gate) and the broadcast multiply is spread across three engines so it hides
under the DMA stream: per u-group of four 1MB tiles, one direct DVE
tensor_mul, two Act-assisted tiles (scalar engine materializes the mlf
broadcast as packed bf16, DVE then runs the mul in 2x_1p mode), and one on
gpsimd. ~47.3MB of DMA traffic per core (4.2 fm + 5.3 lfi + 37.7 store),
~131us of serialized DMA-engine time."""

import numpy as np

import concourse.bass as bass
import concourse.bacc as bacc
import concourse.bass_isa as bass_isa
import concourse.mybir as mybir
import concourse.tile as tile
from concourse.bass_utils import run_bass_kernel_spmd

F32 = mybir.dt.float32
BF16 = mybir.dt.bfloat16
F16 = mybir.dt.float16
FP8 = mybir.dt.float8e4

B, U, H, W, V, N = 4, 9, 256, 256, 9, 64
HY = H // 2


def build_kernel_body(nc, tc, lfi_s, fm, out_s):
    with (
        tc.tile_pool(name="const", bufs=1) as const_pool,
        tc.tile_pool(name="fmp", bufs=1) as fm_pool,
        tc.tile_pool(name="psum", bufs=1, space="PSUM") as psum_pool,
        tc.tile_pool(name="stats", bufs=1) as stats_pool,
        tc.tile_pool(name="lfip", bufs=6) as lfi_pool,
        tc.tile_pool(name="mlfp", bufs=1) as mlf_pool,
        tc.tile_pool(name="outp", bufs=2) as out_pool,
    ):
        ones = const_pool.tile([128, 1], FP8)
        nc.vector.memset(ones[:], 1.0)

        # ---- loads (all on the gpsimd SWDGE queue, which can cast) --------
        # fm in four (w-half, h-half) chunks, f32 -> fp8, so the PE colsum
        # starts after the first 2.9us chunk instead of after the full load
        fmt = {}
        for wc in range(2):
            for h in range(2):
                t = fm_pool.tile([128, HY, N], FP8, name=f"fm{wc}{h}")
                nc.gpsimd.dma_start(
                    out=t[:],
                    in_=fm[h * HY : (h + 1) * HY, wc * HY : (wc + 1) * HY, :],
                )
                fmt[(wc, h)] = t
        # first four lfi u-slices, f32 -> bf16
        lts = []
        for u in range(U):
            lt = lfi_pool.tile([128, W, V], BF16, name=f"lt{u}", tag="lt", bufs=7)
            lts.append(lt)
        for u in range(4):
            nc.gpsimd.dma_start(out=lts[u][:], in_=lfi_s[u])

        # ---- colsum[w, n] = sum_h fm[h, w, n] via fp8 PE ones-matmuls -----
        # one PSUM tile per (w-half, h-half) chunk: self-contained matmuls,
        # h-halves summed on DVE, so the PE starts on chunk 0 right away
        cs_ps = [
            psum_pool.tile([128, N], F32, name=f"cs_ps{i}") for i in range(4)
        ]
        for wc in range(2):
            for h in range(2):
                t = fmt[(wc, h)]
                ps = cs_ps[wc * 2 + h]
                for n in range(N):
                    nc.tensor.matmul(
                        out=ps[:, n : n + 1],
                        lhsT=t[:, :, n],
                        rhs=ones[:, 0:1],
                        start=True,
                        stop=True,
                    )

        # ---- stats: s = sum_w colsum, m = max_w colsum (gpsimd) -----------
        # DVE order: copy csA early, slot the first two lfi reduces in while
        # the PE finishes the B half, then copy csB and finalize.
        mlf = [
            mlf_pool.tile([128, W], F32, name=f"mlf{u}", tag=f"mlf{u}")
            for u in range(U)
        ]

        def reduce_u(u):
            # quarters: the scheduler fills stats-chain bubbles with whatever
            # DVE work is ready -- keep those fillers at 0.6us, not 2.5us
            for q in range(4):
                sl = slice(q * 64, (q + 1) * 64)
                nc.vector.reduce_sum(
                    out=mlf[u][:, sl], in_=lts[u][:, sl, :], axis=mybir.AxisListType.X
                )

        # (DVE may read at most one PSUM operand per instruction)
        tA = stats_pool.tile([128, N], F32)
        nc.vector.tensor_copy(out=tA[:], in_=cs_ps[0][:])
        csA = stats_pool.tile([128, N], F32)
        nc.vector.tensor_add(out=csA[:], in0=tA[:], in1=cs_ps[1][:])
        reduce_u(0)
        reduce_u(1)
        tB = stats_pool.tile([128, N], F32)
        nc.vector.tensor_copy(out=tB[:], in_=cs_ps[2][:])
        csB = stats_pool.tile([128, N], F32)
        nc.vector.tensor_add(out=csB[:], in0=tB[:], in1=cs_ps[3][:])

        with tc.high_priority():
            csS = stats_pool.tile([128, N], F32)
            nc.vector.tensor_add(out=csS[:], in0=csA[:], in1=csB[:])
            csM = stats_pool.tile([128, N], F32)
            nc.vector.tensor_max(out=csM[:], in0=csA[:], in1=csB[:])
            s_all = stats_pool.tile([128, N], F32)
            nc.gpsimd.partition_all_reduce(
                s_all[:], csS[:], 128, bass_isa.ReduceOp.add
            )
            m_all = stats_pool.tile([128, N], F32)
            nc.gpsimd.partition_all_reduce(
                m_all[:], csM[:], 128, bass_isa.ReduceOp.max
            )
        for u in range(4, U):
            nc.gpsimd.dma_start(out=lts[u][:], in_=lfi_s[u])

        # ---- finalize wf[y, n] = colsum[y,n] * s[n] / (max[n] * V) --------
        # high_priority: every store gates on wf
        with tc.high_priority():
            m9 = stats_pool.tile([128, N], F32)
            nc.vector.tensor_scalar_mul(m9[:], m_all[:], float(V))
            rec = stats_pool.tile([128, N], F32)
            nc.vector.reciprocal(out=rec[:], in_=m9[:])
            sn = stats_pool.tile([128, N], F32)
            nc.vector.tensor_mul(out=sn[:], in0=s_all[:], in1=rec[:])
            wf = stats_pool.tile([128, N], F32)
            nc.vector.tensor_mul(out=wf[:], in0=csA[:], in1=sn[:])
            wfb = stats_pool.tile([128, N], F16)
            nc.vector.tensor_copy(out=wfb[:], in_=wf[:])

        # ---- output phase -------------------------------------------------

        xw = 64

        def mlf_bcast(u, x0, xn):
            msl = mlf[u][:, x0 : x0 + xn]
            return bass.AP(
                tensor=msl.tensor, offset=msl.offset, ap=list(msl.ap) + [[0, N]]
            )

        def wf_bcast(t, xn):
            return bass.AP(
                tensor=t.tensor, offset=t.offset, ap=[t.ap[0], [0, xn], t.ap[1]]
            )

        def store(u, x0, xn, ot):
            nc.sync.dma_start(out=out_s[u, :, x0 : x0 + xn, :], in_=ot[:])

        def emit_direct(u, x0, xn=xw):
            ot = out_pool.tile(
                [128, xn, N], BF16, name=f"otd{u}_{x0}", tag=f"otd{xn}", bufs=3
            )
            nc.vector.tensor_mul(
                out=ot[:], in0=mlf_bcast(u, x0, xn), in1=wf_bcast(wf, xn)
            )
            store(u, x0, xn, ot)

        def emit_act_copy(u, x0):
            mr = out_pool.tile([128, xw, N], F16, name=f"mr{u}_{x0}", tag="mr", bufs=3)
            nc.scalar.copy(out=mr[:], in_=mlf_bcast(u, x0, xw))
            return mr

        def emit_act_mul(u, x0, mr):
            ot = out_pool.tile([128, xw, N], BF16, name=f"ota{u}_{x0}", tag="ota", bufs=3)
            nc.vector.tensor_mul(out=ot[:], in0=mr[:], in1=wf_bcast(wfb, xw))
            store(u, x0, xw, ot)

        def emit_pool(u, x0):
            ot = out_pool.tile([128, xw, N], BF16, name=f"otp{u}_{x0}", tag="otp", bufs=3)
            nc.gpsimd.tensor_mul(
                out=ot[:], in0=mlf_bcast(u, x0, xw), in1=wf_bcast(wf, xw)
            )
            store(u, x0, xw, ot)

        for u in range(U):
            mr1 = emit_act_copy(u, 1 * xw)
            mr2 = emit_act_copy(u, 2 * xw)
            emit_pool(u, 3 * xw)
            if u + 2 < U:
                reduce_u(u + 2)
            if u == 0:
                # 0.5MB first tiles so the store stream starts ~1.5us sooner
                emit_direct(0, 0, 32)
                emit_direct(0, 32, 32)
            else:
                emit_direct(u, 0)
            emit_act_mul(u, 1 * xw, mr1)
            emit_act_mul(u, 2 * xw, mr2)


def build_nc():
    nc = bacc.Bacc(
        "TRN2",
        target_bir_lowering=False,
        debug=True,
        # 11 SWDGE loads x 128 descriptors: default 1024-desc ring stalls
        # the lfi stream behind slot-release semaphores
        dynamic_dma_scratch_size=32768,
    )
    lfi_s = nc.dram_tensor("lfi_s", [U, HY, W, V], F32, kind="ExternalInput")
    fm = nc.dram_tensor("fm", [H, W, N], F32, kind="ExternalInput")
    out_s = nc.dram_tensor("out_s", [U, HY, W, N], BF16, kind="ExternalOutput")
    with tile.TileContext(nc) as tc:
        build_kernel_body(nc, tc, lfi_s, fm, out_s)
    nc.compile()
    return nc


_CACHE = {}


def make_in_maps(lfi, f_maps):
    in_maps = []
    for c in range(8):
        b, half = divmod(c, 2)
        lf = np.ascontiguousarray(lfi[b, :, half * HY : (half + 1) * HY])
        fb = f_maps[b]
        if half == 0:
            fmc = np.ascontiguousarray(fb)
        else:
            # rotate W so this core's own colsum rows are the first 128
            fmc = np.ascontiguousarray(
                np.concatenate([fb[:, HY:, :], fb[:, :HY, :]], axis=1)
            )
        in_maps.append({"lfi_s": lf, "fm": fmc})
    return in_maps


def kernel(lfi, f_maps):
    lfi = np.asarray(lfi, dtype=np.float32)
    f_maps = np.asarray(f_maps, dtype=np.float32)
    if "nc" not in _CACHE:
        _CACHE["nc"] = build_nc()
    nc = _CACHE["nc"]
    res = run_bass_kernel_spmd(nc, make_in_maps(lfi, f_maps), list(range(8)))
    out = np.empty((B, U, H, W, N), np.float32)
    for c in range(8):
        b, half = divmod(c, 2)
        out[b, :, half * HY : (half + 1) * HY] = np.asarray(
            res.results[c]["out_s"]
        ).astype(np.float32)
    return out


# revision 23
# speedup vs baseline: 1.1047x; 1.0014x over previous
"""DepthCueExtractor TRN2 kernel.

out[b,u,y,x,n] = mean_v(lfi[b,u,y,x,v]) * s_mask[b,n] * h_mask[b,n,y]
  s_mask[b,n]   = sum_{h,w} f_maps[b,h,w,n]
  h_mask[b,n,y] = colsum[b,y,n] / max_w colsum[b,w,n]
  colsum[b,w,n] = sum_h f_maps[b,h,w,n]

Sharding: 8 cores = (batch b in 0..3) x (H-half in 0..1), data-parallel on the
output; cores are fully independent (no collectives -- the 15us constant
overhead of a collective can no longer hide under the shortened DMA stream).
Each core reads its lfi slice (cast f32->bf16 in the DMA) plus the full
f_maps[b] (cast f32->fp8e4m3 in the DMA; the host rotates the W axis per core
so "my" 128 colsum rows are always the first block, keeping the SPMD program
identical across cores). colsum is reduced over the H partition dim with fp8
PE ones-matmuls; the per-sample stats (sum / max over w) use gpsimd partition
reduces. The output phase writes bf16 (rel-err ~2^-9, far inside the 2e-2
gate) and the broadcast multiply is spread across three engines so it hides
under the DMA stream: per u-group of four 1MB tiles, one direct DVE
tensor_mul, two Act-assisted tiles (scalar engine materializes the mlf
broadcast as packed bf16, DVE then runs the mul in 2x_1p mode), and one on
gpsimd. ~47.3MB of DMA traffic per core (4.2 fm + 5.3 lfi + 37.7 store),
~131us of serialized DMA-engine time."""

import numpy as np

import concourse.bass as bass
import concourse.bacc as bacc
import concourse.bass_isa as bass_isa
import concourse.mybir as mybir
import concourse.tile as tile
from concourse.bass_utils import run_bass_kernel_spmd

F32 = mybir.dt.float32
BF16 = mybir.dt.bfloat16
F16 = mybir.dt.float16
FP8 = mybir.dt.float8e4

B, U, H, W, V, N = 4, 9, 256, 256, 9, 64
HY = H // 2


def build_kernel_body(nc, tc, lfi_s, fm, out_s):
    with (
        tc.tile_pool(name="const", bufs=1) as const_pool,
        tc.tile_pool(name="fmp", bufs=1) as fm_pool,
        tc.tile_pool(name="psum", bufs=1, space="PSUM") as psum_pool,
        tc.tile_pool(name="stats", bufs=1) as stats_pool,
        tc.tile_pool(name="lfip", bufs=6) as lfi_pool,
        tc.tile_pool(name="mlfp", bufs=1) as mlf_pool,
        tc.tile_pool(name="outp", bufs=2) as out_pool,
    ):
        ones = const_pool.tile([128, 1], FP8)
        nc.vector.memset(ones[:], 1.0)

        # ---- loads (all on the gpsimd SWDGE queue, which can cast) --------
        # fm in four (w-half, h-half) chunks, f32 -> fp8, so the PE colsum
        # starts after the first 2.9us chunk instead of after the full load
        fmt = {}
        for wc in range(2):
            for h in range(2):
                t = fm_pool.tile([128, HY, N], FP8, name=f"fm{wc}{h}")
                nc.gpsimd.dma_start(
                    out=t[:],
                    in_=fm[h * HY : (h + 1) * HY, wc * HY : (wc + 1) * HY, :],
                )
                fmt[(wc, h)] = t
        # first four lfi u-slices, f32 -> bf16
        lts = []
        for u in range(U):
            lt = lfi_pool.tile([128, W, V], BF16, name=f"lt{u}", tag="lt", bufs=9)
            lts.append(lt)
        for u in range(U):
            nc.gpsimd.dma_start(out=lts[u][:], in_=lfi_s[u])

        # ---- colsum[w, n] = sum_h fm[h, w, n] via fp8 PE ones-matmuls -----
        # one PSUM tile per (w-half, h-half) chunk: self-contained matmuls,
        # h-halves summed on DVE, so the PE starts on chunk 0 right away
        cs_ps = [
            psum_pool.tile([128, N], F32, name=f"cs_ps{i}") for i in range(4)
        ]
        for wc in range(2):
            for h in range(2):
                t = fmt[(wc, h)]
                ps = cs_ps[wc * 2 + h]
                for n in range(N):
                    nc.tensor.matmul(
                        out=ps[:, n : n + 1],
                        lhsT=t[:, :, n],
                        rhs=ones[:, 0:1],
                        start=True,
                        stop=True,
                    )

        # ---- stats: s = sum_w colsum, m = max_w colsum (gpsimd) -----------
        # DVE order: copy csA early, slot the first two lfi reduces in while
        # the PE finishes the B half, then copy csB and finalize.
        mlf = [
            mlf_pool.tile([128, W], F32, name=f"mlf{u}", tag=f"mlf{u}")
            for u in range(U)
        ]

        def reduce_u(u):
            # quarters: the scheduler fills stats-chain bubbles with whatever
            # DVE work is ready -- keep those fillers at 0.6us, not 2.5us
            for q in range(4):
                sl = slice(q * 64, (q + 1) * 64)
                nc.vector.reduce_sum(
                    out=mlf[u][:, sl], in_=lts[u][:, sl, :], axis=mybir.AxisListType.X
                )

        # (DVE may read at most one PSUM operand per instruction)
        tA = stats_pool.tile([128, N], F32)
        nc.vector.tensor_copy(out=tA[:], in_=cs_ps[0][:])
        csA = stats_pool.tile([128, N], F32)
        nc.vector.tensor_add(out=csA[:], in0=tA[:], in1=cs_ps[1][:])
        reduce_u(0)
        reduce_u(1)
        tB = stats_pool.tile([128, N], F32)
        nc.vector.tensor_copy(out=tB[:], in_=cs_ps[2][:])
        csB = stats_pool.tile([128, N], F32)
        nc.vector.tensor_add(out=csB[:], in0=tB[:], in1=cs_ps[3][:])

        with tc.high_priority():
            csS = stats_pool.tile([128, N], F32)
            nc.vector.tensor_add(out=csS[:], in0=csA[:], in1=csB[:])
            csM = stats_pool.tile([128, N], F32)
            nc.vector.tensor_max(out=csM[:], in0=csA[:], in1=csB[:])
            s_all = stats_pool.tile([128, N], F32)
            nc.gpsimd.partition_all_reduce(
                s_all[:], csS[:], 128, bass_isa.ReduceOp.add
            )
            m_all = stats_pool.tile([128, N], F32)
            nc.gpsimd.partition_all_reduce(
                m_all[:], csM[:], 128, bass_isa.ReduceOp.max
            )

        # ---- finalize wf[y, n] = colsum[y,n] * s[n] / (max[n] * V) --------
        # high_priority: every store gates on wf
        with tc.high_priority():
            m9 = stats_pool.tile([128, N], F32)
            nc.vector.tensor_scalar_mul(m9[:], m_all[:], float(V))
            rec = stats_pool.tile([128, N], F32)
            nc.vector.reciprocal(out=rec[:], in_=m9[:])
            sn = stats_pool.tile([128, N], F32)
            nc.vector.tensor_mul(out=sn[:], in0=s_all[:], in1=rec[:])
            wf = stats_pool.tile([128, N], F32)
            nc.vector.tensor_mul(out=wf[:], in0=csA[:], in1=sn[:])
            wfb = stats_pool.tile([128, N], F16)
            nc.vector.tensor_copy(out=wfb[:], in_=wf[:])

        # ---- output phase -------------------------------------------------

        xw = 64

        def mlf_bcast(u, x0, xn):
            msl = mlf[u][:, x0 : x0 + xn]
            return bass.AP(
                tensor=msl.tensor, offset=msl.offset, ap=list(msl.ap) + [[0, N]]
            )

        def wf_bcast(t, xn):
            return bass.AP(
                tensor=t.tensor, offset=t.offset, ap=[t.ap[0], [0, xn], t.ap[1]]
            )

        def store(u, x0, xn, ot):
            nc.sync.dma_start(out=out_s[u, :, x0 : x0 + xn, :], in_=ot[:])

        def emit_direct(u, x0, xn=xw):
            ot = out_pool.tile(
                [128, xn, N], BF16, name=f"otd{u}_{x0}", tag=f"otd{xn}", bufs=2 if xn == 32 else 3
            )
            nc.vector.tensor_mul(
                out=ot[:], in0=mlf_bcast(u, x0, xn), in1=wf_bcast(wf, xn)
            )
            store(u, x0, xn, ot)

        def emit_act_copy(u, x0):
            mr = out_pool.tile([128, xw, N], F16, name=f"mr{u}_{x0}", tag="mr", bufs=3)
            nc.scalar.copy(out=mr[:], in_=mlf_bcast(u, x0, xw))
            return mr

        def emit_act_mul(u, x0, mr):
            ot = out_pool.tile([128, xw, N], BF16, name=f"ota{u}_{x0}", tag="ota", bufs=3)
            nc.vector.tensor_mul(out=ot[:], in0=mr[:], in1=wf_bcast(wfb, xw))
            store(u, x0, xw, ot)

        def emit_pool_mult(u, x0):
            ot = out_pool.tile([128, xw, N], BF16, name=f"otp{u}_{x0}", tag="otp", bufs=3)
            nc.gpsimd.tensor_mul(
                out=ot[:], in0=mlf_bcast(u, x0, xw), in1=wf_bcast(wf, xw)
            )
            return ot

        # store order per block: D, A, A, then the PREVIOUS block's Pool tile
        # (Pool tiles take 8.2us -- enqueued at their own block's position
        # they head-of-line-block the store queue)
        pend_pool = None
        for u in range(U):
            mr1 = emit_act_copy(u, 1 * xw)
            mr2 = emit_act_copy(u, 2 * xw)
            otp = emit_pool_mult(u, 3 * xw)
            if u == 0:
                # 0.5MB first tiles so the store stream starts ~1.5us sooner
                emit_direct(0, 0, 32)
                emit_direct(0, 32, 32)
            else:
                emit_direct(u, 0)
            emit_act_mul(u, 1 * xw, mr1)
            emit_act_mul(u, 2 * xw, mr2)
            if pend_pool is not None:
                store(*pend_pool)
            pend_pool = (u, 3 * xw, xw, otp)
            # reduces emitted after the mults: the scheduler follows emission
            # order for equally-ready DVE work, and mults gate stores
            if u + 2 < U:
                reduce_u(u + 2)
        store(*pend_pool)


def build_nc():
    nc = bacc.Bacc(
        "TRN2",
        target_bir_lowering=False,
        debug=True,
        # 11 SWDGE loads x 128 descriptors: default 1024-desc ring stalls
        # the lfi stream behind slot-release semaphores
        dynamic_dma_scratch_size=32768,
    )
    lfi_s = nc.dram_tensor("lfi_s", [U, HY, W, V], F32, kind="ExternalInput")
    fm = nc.dram_tensor("fm", [H, W, N], F32, kind="ExternalInput")
    out_s = nc.dram_tensor("out_s", [U, HY, W, N], BF16, kind="ExternalOutput")
    with tile.TileContext(nc) as tc:
        build_kernel_body(nc, tc, lfi_s, fm, out_s)
    nc.compile()
    return nc


_CACHE = {}


def make_in_maps(lfi, f_maps):
    in_maps = []
    for c in range(8):
        b, half = divmod(c, 2)
        lf = np.ascontiguousarray(lfi[b, :, half * HY : (half + 1) * HY])
        fb = f_maps[b]
        if half == 0:
            fmc = np.ascontiguousarray(fb)
        else:
            # rotate W so this core's own colsum rows are the first 128
            fmc = np.ascontiguousarray(
                np.concatenate([fb[:, HY:, :], fb[:, :HY, :]], axis=1)
            )
        in_maps.append({"lfi_s": lf, "fm": fmc})
    return in_maps


def kernel(lfi, f_maps):
    lfi = np.asarray(lfi, dtype=np.float32)
    f_maps = np.asarray(f_maps, dtype=np.float32)
    if "nc" not in _CACHE:
        _CACHE["nc"] = build_nc()
    nc = _CACHE["nc"]
    res = run_bass_kernel_spmd(nc, make_in_maps(lfi, f_maps), list(range(8)))
    out = np.empty((B, U, H, W, N), np.float32)
    for c in range(8):
        b, half = divmod(c, 2)
        out[b, :, half * HY : (half + 1) * HY] = np.asarray(
            res.results[c]["out_s"]
        ).astype(np.float32)
    return out


# revision 29
# speedup vs baseline: 1.1076x; 1.0027x over previous
"""DepthCueExtractor TRN2 kernel.

out[b,u,y,x,n] = mean_v(lfi[b,u,y,x,v]) * s_mask[b,n] * h_mask[b,n,y]
  s_mask[b,n]   = sum_{h,w} f_maps[b,h,w,n]
  h_mask[b,n,y] = colsum[b,y,n] / max_w colsum[b,w,n]
  colsum[b,w,n] = sum_h f_maps[b,h,w,n]

Sharding: 8 cores = (batch b in 0..3) x (H-half in 0..1), data-parallel on the
output; cores are fully independent (no collectives -- the 15us constant
overhead of a collective can no longer hide under the shortened DMA stream).
Each core reads its lfi slice (cast f32->bf16 in the DMA) plus the full
f_maps[b] (cast f32->fp8e4m3 in the DMA; the host rotates the W axis per core
so "my" 128 colsum rows are always the first block, keeping the SPMD program
identical across cores). colsum is reduced over the H partition dim with fp8
PE ones-matmuls; the per-sample stats (sum / max over w) use gpsimd partition
reduces. The output phase writes bf16 (rel-err ~2^-9, far inside the 2e-2
gate) and the broadcast multiply is spread across three engines so it hides
under the DMA stream: per u-group of four 1MB tiles, one direct DVE
tensor_mul, two Act-assisted tiles (scalar engine materializes the mlf
broadcast as packed bf16, DVE then runs the mul in 2x_1p mode), and one on
gpsimd. ~47.3MB of DMA traffic per core (4.2 fm + 5.3 lfi + 37.7 store),
~131us of serialized DMA-engine time."""

import numpy as np

import concourse.bass as bass
import concourse.bacc as bacc
import concourse.bass_isa as bass_isa
import concourse.mybir as mybir
import concourse.tile as tile
from concourse.bass_utils import run_bass_kernel_spmd

F32 = mybir.dt.float32
BF16 = mybir.dt.bfloat16
F16 = mybir.dt.float16
FP8 = mybir.dt.float8e4

B, U, H, W, V, N = 4, 9, 256, 256, 9, 64
HY = H // 2


def build_kernel_body(nc, tc, lfi_s, fm, out_s):
    with (
        tc.tile_pool(name="const", bufs=1) as const_pool,
        tc.tile_pool(name="fmp", bufs=1) as fm_pool,
        tc.tile_pool(name="psum", bufs=1, space="PSUM") as psum_pool,
        tc.tile_pool(name="stats", bufs=1) as stats_pool,
        tc.tile_pool(name="lfip", bufs=6) as lfi_pool,
        tc.tile_pool(name="mlfp", bufs=1) as mlf_pool,
        tc.tile_pool(name="outp", bufs=2) as out_pool,
    ):
        ones = const_pool.tile([128, 1], FP8)
        nc.vector.memset(ones[:], 1.0)

        # ---- loads (all on the gpsimd SWDGE queue, which can cast) --------
        # fm in four (w-half, h-half) chunks, f32 -> fp8, so the PE colsum
        # starts after the first 2.9us chunk instead of after the full load
        fmt = {}
        for wc in range(2):
            for h in range(2):
                t = fm_pool.tile([128, HY, N], FP8, name=f"fm{wc}{h}")
                nc.gpsimd.dma_start(
                    out=t[:],
                    in_=fm[h * HY : (h + 1) * HY, wc * HY : (wc + 1) * HY, :],
                )
                fmt[(wc, h)] = t
        # first four lfi u-slices, f32 -> bf16
        lts = []
        for u in range(U):
            lt = lfi_pool.tile([128, W, V], BF16, name=f"lt{u}", tag="lt", bufs=9)
            lts.append(lt)
        for u in range(U):
            nc.gpsimd.dma_start(out=lts[u][:], in_=lfi_s[u])

        # ---- colsum[w, n] = sum_h fm[h, w, n] via fp8 PE ones-matmuls -----
        # one PSUM tile per (w-half, h-half) chunk: self-contained matmuls,
        # h-halves summed on DVE, so the PE starts on chunk 0 right away
        cs_ps = [
            psum_pool.tile([128, N], F32, name=f"cs_ps{i}") for i in range(4)
        ]
        for wc in range(2):
            for h in range(2):
                t = fmt[(wc, h)]
                ps = cs_ps[wc * 2 + h]
                for n in range(N):
                    nc.tensor.matmul(
                        out=ps[:, n : n + 1],
                        lhsT=t[:, :, n],
                        rhs=ones[:, 0:1],
                        start=True,
                        stop=True,
                    )

        # ---- stats: s = sum_w colsum, m = max_w colsum (gpsimd) -----------
        # DVE order: copy csA early, slot the first two lfi reduces in while
        # the PE finishes the B half, then copy csB and finalize.
        mlf = [
            mlf_pool.tile([128, W], F32, name=f"mlf{u}", tag=f"mlf{u}")
            for u in range(U)
        ]

        def reduce_u(u):
            # quarters: the scheduler fills stats-chain bubbles with whatever
            # DVE work is ready -- keep those fillers at 0.6us, not 2.5us
            for q in range(4):
                sl = slice(q * 64, (q + 1) * 64)
                nc.vector.reduce_sum(
                    out=mlf[u][:, sl], in_=lts[u][:, sl, :], axis=mybir.AxisListType.X
                )

        # (DVE may read at most one PSUM operand per instruction)
        tA = stats_pool.tile([128, N], F32)
        nc.vector.tensor_copy(out=tA[:], in_=cs_ps[0][:])
        csA = stats_pool.tile([128, N], F32)
        nc.vector.tensor_add(out=csA[:], in0=tA[:], in1=cs_ps[1][:])
        reduce_u(0)
        reduce_u(1)
        tB = stats_pool.tile([128, N], F32)
        nc.vector.tensor_copy(out=tB[:], in_=cs_ps[2][:])
        csB = stats_pool.tile([128, N], F32)
        nc.vector.tensor_add(out=csB[:], in0=tB[:], in1=cs_ps[3][:])

        with tc.high_priority():
            csS = stats_pool.tile([128, N], F32)
            nc.vector.tensor_add(out=csS[:], in0=csA[:], in1=csB[:])
            csM = stats_pool.tile([128, N], F32)
            nc.vector.tensor_max(out=csM[:], in0=csA[:], in1=csB[:])
            s_all = stats_pool.tile([128, N], F32)
            nc.gpsimd.partition_all_reduce(
                s_all[:], csS[:], 128, bass_isa.ReduceOp.add
            )
            m_all = stats_pool.tile([128, N], F32)
            nc.gpsimd.partition_all_reduce(
                m_all[:], csM[:], 128, bass_isa.ReduceOp.max
            )

        # ---- finalize wf[y, n] = colsum[y,n] * s[n] / (max[n] * V) --------
        # high_priority: every store gates on wf
        with tc.high_priority():
            m9 = stats_pool.tile([128, N], F32)
            nc.vector.tensor_scalar_mul(m9[:], m_all[:], float(V))
            rec = stats_pool.tile([128, N], F32)
            nc.vector.reciprocal(out=rec[:], in_=m9[:])
            sn = stats_pool.tile([128, N], F32)
            nc.vector.tensor_mul(out=sn[:], in0=s_all[:], in1=rec[:])
            wf = stats_pool.tile([128, N], F32)
            nc.vector.tensor_mul(out=wf[:], in0=csA[:], in1=sn[:])
            wfb = stats_pool.tile([128, N], F16)
            nc.vector.tensor_copy(out=wfb[:], in_=wf[:])

        # ---- output phase -------------------------------------------------

        xw = 64

        def mlf_bcast(u, x0, xn):
            msl = mlf[u][:, x0 : x0 + xn]
            return bass.AP(
                tensor=msl.tensor, offset=msl.offset, ap=list(msl.ap) + [[0, N]]
            )

        def wf_bcast(t, xn):
            return bass.AP(
                tensor=t.tensor, offset=t.offset, ap=[t.ap[0], [0, xn], t.ap[1]]
            )

        def store(u, x0, xn, ot):
            nc.sync.dma_start(out=out_s[u, :, x0 : x0 + xn, :], in_=ot[:])

        def emit_direct(u, x0, xn=xw):
            ot = out_pool.tile(
                [128, xn, N], BF16, name=f"otd{u}_{x0}", tag=f"otd{xn}", bufs=2 if xn == 32 else 3
            )
            nc.vector.tensor_mul(
                out=ot[:], in0=mlf_bcast(u, x0, xn), in1=wf_bcast(wf, xn)
            )
            store(u, x0, xn, ot)

        def emit_act_copy(u, x0):
            mr = out_pool.tile([128, xw, N], F16, name=f"mr{u}_{x0}", tag="mr", bufs=3)
            nc.scalar.copy(out=mr[:], in_=mlf_bcast(u, x0, xw))
            return mr

        def emit_act_mul(u, x0, mr):
            ot = out_pool.tile([128, xw, N], BF16, name=f"ota{u}_{x0}", tag="ota", bufs=3)
            nc.vector.tensor_mul(out=ot[:], in0=mr[:], in1=wf_bcast(wfb, xw))
            store(u, x0, xw, ot)

        def emit_pool_mult(u, x0):
            ot = out_pool.tile([128, xw, N], BF16, name=f"otp{u}_{x0}", tag="otp", bufs=3)
            nc.gpsimd.tensor_mul(
                out=ot[:], in0=mlf_bcast(u, x0, xw), in1=wf_bcast(wf, xw)
            )
            return ot

        # store order per block: D, A, A, then the PREVIOUS block's Pool tile
        # (Pool tiles take 8.2us -- enqueued at their own block's position
        # they head-of-line-block the store queue)
        pend_pool = None
        for u in range(U):
            mr0 = emit_act_copy(u, 0) if u > 0 else None
            mr1 = emit_act_copy(u, 1 * xw)
            mr2 = emit_act_copy(u, 2 * xw)
            otp = emit_pool_mult(u, 3 * xw)
            if u == 0:
                # 0.5MB first tiles so the store stream starts ~1.5us sooner
                emit_direct(0, 0, 32)
                emit_direct(0, 32, 32)
            else:
                emit_act_mul(u, 0, mr0)
            emit_act_mul(u, 1 * xw, mr1)
            emit_act_mul(u, 2 * xw, mr2)
            if pend_pool is not None:
                store(*pend_pool)
            pend_pool = (u, 3 * xw, xw, otp)
            # reduces emitted after the mults: the scheduler follows emission
            # order for equally-ready DVE work, and mults gate stores
            if u + 2 < U:
                reduce_u(u + 2)
        store(*pend_pool)


def build_nc():
    nc = bacc.Bacc(
        "TRN2",
        target_bir_lowering=False,
        debug=True,
        # 11 SWDGE loads x 128 descriptors: default 1024-desc ring stalls
        # the lfi stream behind slot-release semaphores
        dynamic_dma_scratch_size=32768,
    )
    lfi_s = nc.dram_tensor("lfi_s", [U, HY, W, V], F32, kind="ExternalInput")
    fm = nc.dram_tensor("fm", [H, W, N], F32, kind="ExternalInput")
    out_s = nc.dram_tensor("out_s", [U, HY, W, N], BF16, kind="ExternalOutput")
    with tile.TileContext(nc) as tc:
        build_kernel_body(nc, tc, lfi_s, fm, out_s)
    nc.compile()
    return nc


_CACHE = {}


def make_in_maps(lfi, f_maps):
    in_maps = []
    for c in range(8):
        b, half = divmod(c, 2)
        lf = np.ascontiguousarray(lfi[b, :, half * HY : (half + 1) * HY])
        fb = f_maps[b]
        if half == 0:
            fmc = np.ascontiguousarray(fb)
        else:
            # rotate W so this core's own colsum rows are the first 128
            fmc = np.ascontiguousarray(
                np.concatenate([fb[:, HY:, :], fb[:, :HY, :]], axis=1)
            )
        in_maps.append({"lfi_s": lf, "fm": fmc})
    return in_maps


def kernel(lfi, f_maps):
    lfi = np.asarray(lfi, dtype=np.float32)
    f_maps = np.asarray(f_maps, dtype=np.float32)
    if "nc" not in _CACHE:
        _CACHE["nc"] = build_nc()
    nc = _CACHE["nc"]
    res = run_bass_kernel_spmd(nc, make_in_maps(lfi, f_maps), list(range(8)))
    out = np.empty((B, U, H, W, N), np.float32)
    for c in range(8):
        b, half = divmod(c, 2)
        out[b, :, half * HY : (half + 1) * HY] = np.asarray(
            res.results[c]["out_s"]
        ).astype(np.float32)
    return out


# revision 33
# speedup vs baseline: 1.1138x; 1.0057x over previous
"""DepthCueExtractor TRN2 kernel.

out[b,u,y,x,n] = mean_v(lfi[b,u,y,x,v]) * s_mask[b,n] * h_mask[b,n,y]
  s_mask[b,n]   = sum_{h,w} f_maps[b,h,w,n]
  h_mask[b,n,y] = colsum[b,y,n] / max_w colsum[b,w,n]
  colsum[b,w,n] = sum_h f_maps[b,h,w,n]

Sharding: 8 cores = (batch b in 0..3) x (H-half in 0..1), data-parallel on the
output; cores are fully independent (no collectives -- the 15us constant
overhead of a collective can no longer hide under the shortened DMA stream).
Each core reads its lfi slice (cast f32->bf16 in the DMA) plus the full
f_maps[b] (cast f32->fp8e4m3 in the DMA; the host rotates the W axis per core
so "my" 128 colsum rows are always the first block, keeping the SPMD program
identical across cores). colsum is reduced over the H partition dim with fp8
PE ones-matmuls; the per-sample stats (sum / max over w) use gpsimd partition
reduces. The output phase writes bf16 (rel-err ~2^-9, far inside the 2e-2
gate) and the broadcast multiply is spread across three engines so it hides
under the DMA stream: per u-group of four 1MB tiles, one direct DVE
tensor_mul, two Act-assisted tiles (scalar engine materializes the mlf
broadcast as packed bf16, DVE then runs the mul in 2x_1p mode), and one on
gpsimd. ~47.3MB of DMA traffic per core (4.2 fm + 5.3 lfi + 37.7 store),
~131us of serialized DMA-engine time."""

import numpy as np

import concourse.bass as bass
import concourse.bacc as bacc
import concourse.bass_isa as bass_isa
import concourse.mybir as mybir
import concourse.tile as tile
from concourse.bass_utils import run_bass_kernel_spmd

F32 = mybir.dt.float32
BF16 = mybir.dt.bfloat16
F16 = mybir.dt.float16
FP8 = mybir.dt.float8e4

B, U, H, W, V, N = 4, 9, 256, 256, 9, 64
HY = H // 2


def build_kernel_body(nc, tc, lfi_s, fm, out_s):
    with (
        tc.tile_pool(name="const", bufs=1) as const_pool,
        tc.tile_pool(name="fmp", bufs=1) as fm_pool,
        tc.tile_pool(name="psum", bufs=1, space="PSUM") as psum_pool,
        tc.tile_pool(name="stats", bufs=1) as stats_pool,
        tc.tile_pool(name="lfip", bufs=6) as lfi_pool,
        tc.tile_pool(name="mlfp", bufs=1) as mlf_pool,
        tc.tile_pool(name="outp", bufs=2) as out_pool,
    ):
        ones = const_pool.tile([128, 1], FP8)
        nc.vector.memset(ones[:], 1.0)

        # ---- loads (all on the gpsimd SWDGE queue, which can cast) --------
        # fm in four (w-half, h-half) chunks, f32 -> fp8, so the PE colsum
        # starts after the first 2.9us chunk instead of after the full load
        fmt = {}
        for wc in range(2):
            for h in range(2):
                t = fm_pool.tile([128, HY, N], FP8, name=f"fm{wc}{h}")
                nc.gpsimd.dma_start(
                    out=t[:],
                    in_=fm[h * HY : (h + 1) * HY, wc * HY : (wc + 1) * HY, :],
                )
                fmt[(wc, h)] = t
        # lfi in two big cast-DMAs (u 0-3 and u 4-8), f32 -> bf16: six total
        # descriptor-gens cover the whole load stream, so no gen is left to
        # stall behind the stat-reduce dispatch on the in-order Pool engine
        YV = W * V
        UYV = HY * YV
        groups = ((0, 1), (1, 3), (4, 3), (7, 2))
        lfi_ap = lfi_s[:]
        lts = []
        for u0, un in groups:
            t = lfi_pool.tile([128, un, W, V], BF16, name=f"lt_{u0}", bufs=1)
            src = bass.AP(
                tensor=lfi_ap.tensor,
                offset=lfi_ap.offset + u0 * UYV,
                ap=[[YV, 128], [UYV, un], [1, YV]],
            )
            nc.gpsimd.dma_start(out=t[:], in_=src)
            lts.extend(t[:, i] for i in range(un))

        # ---- colsum[w, n] = sum_h fm[h, w, n] via fp8 PE ones-matmuls -----
        # one PSUM tile per (w-half, h-half) chunk: self-contained matmuls,
        # h-halves summed on DVE, so the PE starts on chunk 0 right away
        cs_ps = [
            psum_pool.tile([128, N], F32, name=f"cs_ps{i}") for i in range(4)
        ]
        for wc in range(2):
            for h in range(2):
                t = fmt[(wc, h)]
                ps = cs_ps[wc * 2 + h]
                for n in range(N):
                    nc.tensor.matmul(
                        out=ps[:, n : n + 1],
                        lhsT=t[:, :, n],
                        rhs=ones[:, 0:1],
                        start=True,
                        stop=True,
                    )

        # ---- stats: s = sum_w colsum, m = max_w colsum (gpsimd) -----------
        # DVE order: copy csA early, slot the first two lfi reduces in while
        # the PE finishes the B half, then copy csB and finalize.
        mlf = [
            mlf_pool.tile([128, W], F32, name=f"mlf{u}", tag=f"mlf{u}")
            for u in range(U)
        ]

        def reduce_u(u):
            # quarters: the scheduler fills stats-chain bubbles with whatever
            # DVE work is ready -- keep those fillers at 0.6us, not 2.5us
            for q in range(4):
                sl = slice(q * 64, (q + 1) * 64)
                nc.vector.reduce_sum(
                    out=mlf[u][:, sl], in_=lts[u][:, sl, :], axis=mybir.AxisListType.X
                )

        # (DVE may read at most one PSUM operand per instruction)
        tA = stats_pool.tile([128, N], F32)
        nc.vector.tensor_copy(out=tA[:], in_=cs_ps[0][:])
        csA = stats_pool.tile([128, N], F32)
        nc.vector.tensor_add(out=csA[:], in0=tA[:], in1=cs_ps[1][:])
        reduce_u(0)
        reduce_u(1)
        tB = stats_pool.tile([128, N], F32)
        nc.vector.tensor_copy(out=tB[:], in_=cs_ps[2][:])
        csB = stats_pool.tile([128, N], F32)
        nc.vector.tensor_add(out=csB[:], in0=tB[:], in1=cs_ps[3][:])

        with tc.high_priority():
            csS = stats_pool.tile([128, N], F32)
            nc.vector.tensor_add(out=csS[:], in0=csA[:], in1=csB[:])
            csM = stats_pool.tile([128, N], F32)
            nc.vector.tensor_max(out=csM[:], in0=csA[:], in1=csB[:])
            s_all = stats_pool.tile([128, N], F32)
            nc.gpsimd.partition_all_reduce(
                s_all[:], csS[:], 128, bass_isa.ReduceOp.add
            )
            m_all = stats_pool.tile([128, N], F32)
            nc.gpsimd.partition_all_reduce(
                m_all[:], csM[:], 128, bass_isa.ReduceOp.max
            )

        # ---- finalize wf[y, n] = colsum[y,n] * s[n] / (max[n] * V) --------
        # high_priority: every store gates on wf
        with tc.high_priority():
            m9 = stats_pool.tile([128, N], F32)
            nc.vector.tensor_scalar_mul(m9[:], m_all[:], float(V))
            rec = stats_pool.tile([128, N], F32)
            nc.vector.reciprocal(out=rec[:], in_=m9[:])
            sn = stats_pool.tile([128, N], F32)
            nc.vector.tensor_mul(out=sn[:], in0=s_all[:], in1=rec[:])
            wf = stats_pool.tile([128, N], F32)
            nc.vector.tensor_mul(out=wf[:], in0=csA[:], in1=sn[:])
            wfb = stats_pool.tile([128, N], F16)
            nc.vector.tensor_copy(out=wfb[:], in_=wf[:])

        # ---- output phase -------------------------------------------------

        xw = 64

        def mlf_bcast(u, x0, xn):
            msl = mlf[u][:, x0 : x0 + xn]
            return bass.AP(
                tensor=msl.tensor, offset=msl.offset, ap=list(msl.ap) + [[0, N]]
            )

        def wf_bcast(t, xn):
            return bass.AP(
                tensor=t.tensor, offset=t.offset, ap=[t.ap[0], [0, xn], t.ap[1]]
            )

        def store(u, x0, xn, ot):
            nc.sync.dma_start(out=out_s[u, :, x0 : x0 + xn, :], in_=ot[:])

        def emit_direct(u, x0, xn=xw):
            ot = out_pool.tile(
                [128, xn, N], BF16, name=f"otd{u}_{x0}", tag=f"otd{xn}", bufs=2 if xn == 32 else 3
            )
            nc.vector.tensor_mul(
                out=ot[:], in0=mlf_bcast(u, x0, xn), in1=wf_bcast(wf, xn)
            )
            store(u, x0, xn, ot)

        def emit_act_copy(u, x0):
            mr = out_pool.tile([128, xw, N], F16, name=f"mr{u}_{x0}", tag="mr", bufs=3)
            nc.scalar.copy(out=mr[:], in_=mlf_bcast(u, x0, xw))
            return mr

        def emit_act_mul(u, x0, mr):
            ot = out_pool.tile([128, xw, N], BF16, name=f"ota{u}_{x0}", tag="ota", bufs=3)
            nc.vector.tensor_mul(out=ot[:], in0=mr[:], in1=wf_bcast(wfb, xw))
            store(u, x0, xw, ot)

        def emit_pool_mult(u, x0):
            ot = out_pool.tile([128, xw, N], BF16, name=f"otp{u}_{x0}", tag="otp", bufs=3)
            nc.gpsimd.tensor_mul(
                out=ot[:], in0=mlf_bcast(u, x0, xw), in1=wf_bcast(wf, xw)
            )
            return ot

        # store order per block: D, A, A, then the PREVIOUS block's Pool tile
        # (Pool tiles take 8.2us -- enqueued at their own block's position
        # they head-of-line-block the store queue)
        pend_pool = None
        for u in range(U):
            mr0 = emit_act_copy(u, 0) if u > 0 else None
            mr1 = emit_act_copy(u, 1 * xw)
            mr2 = emit_act_copy(u, 2 * xw)
            otp = emit_pool_mult(u, 3 * xw)
            if u == 0:
                # 0.5MB first tiles so the store stream starts ~1.5us sooner
                emit_direct(0, 0, 32)
                emit_direct(0, 32, 32)
            else:
                emit_act_mul(u, 0, mr0)
            emit_act_mul(u, 1 * xw, mr1)
            emit_act_mul(u, 2 * xw, mr2)
            if pend_pool is not None:
                store(*pend_pool)
            pend_pool = (u, 3 * xw, xw, otp)
            # reduces emitted after the mults: the scheduler follows emission
            # order for equally-ready DVE work, and mults gate stores
            if u + 2 < U:
                reduce_u(u + 2)
        store(*pend_pool)


def build_nc():
    nc = bacc.Bacc(
        "TRN2",
        target_bir_lowering=False,
        debug=True,
        # 11 SWDGE loads x 128 descriptors: default 1024-desc ring stalls
        # the lfi stream behind slot-release semaphores
        dynamic_dma_scratch_size=32768,
    )
    lfi_s = nc.dram_tensor("lfi_s", [U, HY, W, V], F32, kind="ExternalInput")
    fm = nc.dram_tensor("fm", [H, W, N], F32, kind="ExternalInput")
    out_s = nc.dram_tensor("out_s", [U, HY, W, N], BF16, kind="ExternalOutput")
    with tile.TileContext(nc) as tc:
        build_kernel_body(nc, tc, lfi_s, fm, out_s)
    nc.compile()
    return nc


_CACHE = {}


def make_in_maps(lfi, f_maps):
    in_maps = []
    for c in range(8):
        b, half = divmod(c, 2)
        lf = np.ascontiguousarray(lfi[b, :, half * HY : (half + 1) * HY])
        fb = f_maps[b]
        if half == 0:
            fmc = np.ascontiguousarray(fb)
        else:
            # rotate W so this core's own colsum rows are the first 128
            fmc = np.ascontiguousarray(
                np.concatenate([fb[:, HY:, :], fb[:, :HY, :]], axis=1)
            )
        in_maps.append({"lfi_s": lf, "fm": fmc})
    return in_maps


def kernel(lfi, f_maps):
    lfi = np.asarray(lfi, dtype=np.float32)
    f_maps = np.asarray(f_maps, dtype=np.float32)
    if "nc" not in _CACHE:
        _CACHE["nc"] = build_nc()
    nc = _CACHE["nc"]
    res = run_bass_kernel_spmd(nc, make_in_maps(lfi, f_maps), list(range(8)))
    out = np.empty((B, U, H, W, N), np.float32)
    for c in range(8):
        b, half = divmod(c, 2)
        out[b, :, half * HY : (half + 1) * HY] = np.asarray(
            res.results[c]["out_s"]
        ).astype(np.float32)
    return out
